# revision 1
# baseline (speedup 1.0000x reference)
"""Trainium2 Bass kernel for CDRExtractor (segment_reduce).

Input : segmentation_mask (64, 3, 512, 512) fp32
Output: (64, 5) fp32 = [cdr, disc_mean, cup_mean, disc_mean, cup_mean]

Sharding: pure data parallel, 8 samples per core across 8 cores; each core
streams its 24 MiB shard once (DMA roofline ~70us/core at ~358 GB/s).

Per-core algorithm (t-space formulation; 16 tiles of 2 samples x 128 rows):
  T = [x1-x0 | x2-x0]                 (POOL TT subtract - only add/sub/mult
                                       are walrus-legal on the Pool engine)
  F = exp(T)  (f0 == 1 implicitly)    (ACT, bf16 out)
  sadd = f1+f2                        (POOL)
  L = ln(1 + sadd); r = exp(-L)       (ACT; +1 via free activation bias.
                                       ACT Reciprocal/Rsqrt are banned; exp
                                       and ln share one act-table set)
  p-sums  Sum_w f*r                   (mostly DVE fused scalar_tensor_
                                       tensor w/ accum_out; 1 per tile (0
                                       on two tiles) via ACT exp(t-L) with
                                       fused fp32 accum_out, tuned per-tile
                                       so ACT/DVE/POOL busy are equal)
  d-counts Sum_w [f > max(f_oth,1)]   (DVE STT is_gt w/ accum; max(f,1)
                                       clamps via POOL tensor_scalar_max;
                                       count>0.5 == row contains
                                       argmax==label, exact)
  tail: PE transpose + ones-matmul over the (128,32) accumulators,
        iota+penalty reduce-min/max for ymin/ymax per (sample,label),
        heights = relu(ymax-ymin), cdr = h_cup/(h_disc+1e-6), means /= H*W.

Engine busy per core (CoreSim cost model): ACT ~76us, DVE ~73us, DMA ~76us,
Pool ~77us - all four at the memory roofline; end-to-end sim/HW-model 95.0us.
STT/TensorReduce have no 2x uop (1 elem/cycle); Pool accepts TT add/sub/
mult and tensor_scalar but rejects TT max/is_gt/STT/reduce at codegen,
which is what fixes this split. Fill is
minimized by a warm-up activation (act-table load at t~0), per-channel
DMAs for tile 0, and deferring const DMAs to the tail. HW-verified
rel err vs reference: 4.5e-05.
"""

import numpy as np
from contextlib import ExitStack

B, C, H, W = 64, 3, 512, 512
NCORES = 8
SPC = B // NCORES      # samples per core = 8
PAIRS = SPC // 2       # sample pairs per core = 4
NB = H // 128          # 128-row blocks = 4
HW = float(H * W)

_CACHE = {}


def _build():
    import concourse.bass as bass
    import concourse.bacc as bacc
    import concourse.mybir as mybir
    from concourse.tile import TileContext

    # Offer only the act-table set containing BOTH exp and ln (ids kept
    # aligned with act_info.json) so the table never reloads mid-kernel.
    if not _CACHE.get("act_patch"):
        _orig_tables = bacc.get_activation_tables

        def _only_ln_exp(arch):
            t = _orig_tables(arch)
            keep = "natural_log_exp_and_others"
            return {k: (v if k == keep else set()) for k, v in t.items()}

        bacc.get_activation_tables = _only_ln_exp
        _CACHE["act_patch"] = True

    f32 = mybir.dt.float32
    bf16 = mybir.dt.bfloat16
    Alu = mybir.AluOpType
    AFT = mybir.ActivationFunctionType
    X_AX = mybir.AxisListType.X

    nc = bacc.Bacc()
    x = nc.dram_tensor("x", (SPC, C, H, W), f32, kind="ExternalInput")
    iota_in = nc.dram_tensor("iota", (32, 128), f32, kind="ExternalInput")
    ident_in = nc.dram_tensor("ident", (128, 128), f32, kind="ExternalInput")
    ones_in = nc.dram_tensor("ones", (128, 1), f32, kind="ExternalInput")
    out = nc.dram_tensor("out", (5, SPC), f32, kind="ExternalOutput")

    with TileContext(nc) as tc, ExitStack() as ctx:
        cpool = ctx.enter_context(tc.tile_pool(name="consts", bufs=1))
        apool = ctx.enter_context(tc.tile_pool(name="accs", bufs=1))
        mpool = ctx.enter_context(tc.tile_pool(name="main", bufs=4))
        ppool = ctx.enter_context(tc.tile_pool(name="ps", bufs=1, space="PSUM"))

        # dummy activation on a memset tile: forces the (one-time) act
        # table load to run at t~0 instead of behind the first X DMA
        warm = cpool.tile([1, 16], bf16, tag="warm")
        nc.vector.memset(warm[:, :], 0.0)
        nc.scalar.activation(warm[:, :], warm[:, :], AFT.Exp)

        iota = cpool.tile([32, 128], f32, tag="iota")
        ident = cpool.tile([128, 128], f32, tag="ident")
        ones = cpool.tile([128, 1], f32, tag="ones")

        # accumulators: col j = b*8 + s
        RS1 = apool.tile([128, 32], f32, tag="RS1")  # row-sums of p1 (cup)
        RS2 = apool.tile([128, 32], f32, tag="RS2")  # row-sums of p2 (disc)
        DM1 = apool.tile([128, 32], f32, tag="DM1")  # row-max argmax margin lbl1
        DM2 = apool.tile([128, 32], f32, tag="DM2")

        def stage_a(t, b):
            """DMA the (2 samples x 128 rows x 3ch) tile."""
            X = mpool.tile([128, 2 * C * W], f32, tag="X", name=f"X_{t}_{b}",
                           bufs=5)
            if (t, b) == (0, 0):
                # fill latency: small per-(sample,channel) DMAs
                for si in range(2):
                    for ci in range(C):
                        src = x[2 * t + si, ci, b * 128:(b + 1) * 128, :]
                        off = (si * C + ci) * W
                        nc.sync.dma_start(X[:, off:off + W], src)
                return X
            src = x[2 * t:2 * t + 2, :, b * 128:(b + 1) * 128, :]
            src = src.rearrange("s c h w -> h s c w")
            Xv = X.rearrange("p (s c w) -> p s c w", s=2, c=C)
            nc.sync.dma_start(Xv, src)
            return X

        def stage_b1(t, b, X):
            """t-space: T = [x1-x0 | x2-x0] (POOL), F = exp(T) (ACT),
            sadd = f1+f2 (POOL)."""
            Xv = X.rearrange("p (s c w) -> p s c w", s=2, c=C)
            T32 = mpool.tile([128, 2048], f32, tag="T32",
                             name=f"T32_{t}_{b}", bufs=4)
            Tv = T32.rearrange("p (s l w) -> p s l w", s=2, l=2)
            F = mpool.tile([128, 2048], bf16, tag="F", name=f"F_{t}_{b}",
                           bufs=4)
            Fv = F.rearrange("p (s l w) -> p s l w", s=2, l=2)
            sadd = mpool.tile([128, 1024], bf16, tag="sadd",
                              name=f"sadd_{t}_{b}", bufs=4)
            saddv = sadd.rearrange("p (s w) -> p s w", s=2)
            if (t, b) == (0, 0):
                # per-sample halves: engines start after 3 channel DMAs
                # instead of 6 (pipeline fill)
                for si in range(2):
                    for li in range(2):
                        nc.gpsimd.tensor_tensor(
                            Tv[:, si:si + 1, li, :],
                            Xv[:, si:si + 1, li + 1, :],
                            Xv[:, si:si + 1, 0, :], Alu.subtract)
                    h = slice(si * 1024, (si + 1) * 1024)
                    nc.scalar.activation(F[:, h], T32[:, h], AFT.Exp)
                    nc.gpsimd.tensor_tensor(
                        saddv[:, si:si + 1], Fv[:, si:si + 1, 0, :],
                        Fv[:, si:si + 1, 1, :], Alu.add)
                return T32, F, sadd
            for li in range(2):
                nc.gpsimd.tensor_tensor(
                    Tv[:, :, li, :], Xv[:, :, li + 1, :], Xv[:, :, 0, :],
                    Alu.subtract)
            nc.scalar.activation(F[:, :], T32[:, :], AFT.Exp)
            nc.gpsimd.tensor_tensor(saddv, Fv[:, :, 0, :], Fv[:, :, 1, :],
                                    Alu.add)
            return T32, F, sadd

        def stage_b2a(t, b, T32, F, sadd):
            """L = ln(1 + f1 + f2) and r = exp(-L) (ACT)."""
            lns = mpool.tile([128, 1024], f32, tag="lns",
                             name=f"lns_{t}_{b}", bufs=4)
            nc.scalar.activation(lns[:, :], sadd[:, :], AFT.Ln, bias=1.0)
            rb = mpool.tile([128, 1024], bf16, tag="rb",
                            name=f"rb_{t}_{b}", bufs=4)
            nc.scalar.activation(rb[:, :], lns[:, :], AFT.Exp, scale=-1.0)
            return lns, rb

        def stage_b2b(t, b, T32, F, sadd, lns, rb):
            """p-sums: 1 of 4 via ACT exp(t-L)-with-accum (2 of 4 on a few
            tiles to equalize ACT/DVE busy), rest via DVE STT (f*r);
            argmax counts via DVE STT vs max(f_other, 1)."""
            k2 = False
            # k0 tiles: all 4 p-sums via DVE STT (drops the ACT p-exp on a
            # couple of tiles so ACT dips under the POOL/DMA pace)
            k0 = (4 * t + b) in (5, 10)
            U = mpool.tile([128, 1024], f32, tag="U", name=f"U_{t}_{b}",
                           bufs=2)
            if not k0:
                nc.gpsimd.tensor_tensor(
                    U[:, 0:512], T32[:, 0:512], lns[:, 0:512], Alu.subtract)
            if k2:
                # (si=1, li=0): t-slice at 1024, lns-slice at 512
                nc.gpsimd.tensor_tensor(
                    U[:, 512:1024], T32[:, 1024:1536], lns[:, 512:1024],
                    Alu.subtract)

            # MM = [max(f2,1) | max(f1,1)] per sample (argmax test
            # [f_l > max(f_other, 1)]); cheap 4x tensor_scalar on DVE
            Fv = F.rearrange("p (s l w) -> p s l w", s=2, l=2)
            MM = mpool.tile([128, 2048], bf16, tag="MM", name=f"MM_{t}_{b}",
                            bufs=2)
            MMv = MM.rearrange("p (s l w) -> p s l w", s=2, l=2)
            nc.gpsimd.tensor_scalar_max(MMv[:, :, 0, :], Fv[:, :, 1, :], 1.0)
            nc.gpsimd.tensor_scalar_max(MMv[:, :, 1, :], Fv[:, :, 0, :], 1.0)

            pscr = mpool.tile([128, 2048], bf16, tag="pscr",
                              name=f"pscr_{t}_{b}", bufs=2)
            dscr = mpool.tile([128, 2048], bf16, tag="dscr",
                              name=f"dscr_{t}_{b}", bufs=2)
            for si in range(2):
                s_g = 2 * t + si
                col = b * 8 + s_g
                for li, RS, DM in ((0, RS1, DM1), (1, RS2, DM2)):
                    sl = slice((si * 2 + li) * 512, (si * 2 + li + 1) * 512)
                    if li == 0 and (si == 0 or k2) and not k0:
                        # p-sum via ACT exp with fused fp32 row-sum
                        usl = slice(si * 512, (si + 1) * 512)
                        nc.scalar.activation(
                            pscr[:, sl], U[:, usl], AFT.Exp,
                            accum_out=RS[:, col:col + 1])
                    else:
                        rsl = slice(si * 512, (si + 1) * 512)
                        nc.vector.scalar_tensor_tensor(
                            pscr[:, sl], F[:, sl], 0.0, rb[:, rsl],
                            Alu.add, Alu.mult, accum_out=RS[:, col:col + 1])
                    # argmax presence count (exact): [f_l > max(f_other,1)]
                    nc.vector.scalar_tensor_tensor(
                        dscr[:, sl], F[:, sl], 0.0, MM[:, sl],
                        Alu.add, Alu.is_gt, accum_out=DM[:, col:col + 1])

        # 3-stage software pipeline: ACT->POOL->ACT round trips mean tile
        # i's ln runs after tile i+1's exp, and its U/p/d stage after tile
        # i+2's exp, so no engine waits on a same-tile cross-engine dep.
        tiles = [(t, b) for t in range(PAIRS) for b in range(NB)]
        pend1 = None  # awaiting b2a (ln)
        pend2 = None  # awaiting b2b (U, p-exps, d-counts)
        for i, (t, b) in enumerate(tiles):
            X = stage_a(t, b)
            T32, F, sadd = stage_b1(t, b, X)
            if i == 0:
                # eager first tile: shortest path to getting DVE going
                lns1, rb1 = stage_b2a(t, b, T32, F, sadd)
                stage_b2b(t, b, T32, F, sadd, lns1, rb1)
                continue
            if pend2 is not None:
                stage_b2b(*pend2)
                pend2 = None
            if pend1 is not None:
                lns1, rb1 = stage_b2a(*pend1)
                pend2 = (*pend1, lns1, rb1)
                pend1 = None
            pend1 = (t, b, T32, F, sadd)
        lns1, rb1 = stage_b2a(*pend1)
        if pend2 is not None:
            stage_b2b(*pend2)
        stage_b2b(*pend1, lns1, rb1)

        # ---- tail ----
        # const loads for the tail (emitted late so they don't delay the
        # first X tile on the SP DMA queue)
        nc.sync.dma_start(iota[:, :], iota_in[:, :])
        nc.sync.dma_start(ident[:, :], ident_in[:, :])
        nc.sync.dma_start(ones[:, :], ones_in[:, :])
        O = cpool.tile([1, 40], f32, tag="O")
        S12 = ppool.tile([1, 64], f32, tag="S12")
        nc.tensor.matmul(S12[:, 0:32], ones[:, :], RS1[:, :], start=True, stop=True)
        nc.tensor.matmul(S12[:, 32:64], ones[:, :], RS2[:, :], start=True, stop=True)

        heights = []
        for li, DM in enumerate((DM1, DM2)):
            TD = ppool.tile([32, 128], f32, tag=f"TD{li}")
            nc.tensor.transpose(TD[:, :], DM[:, :], ident[:, :])
            TL = cpool.tile([32, 128], f32, tag=f"TL{li}")
            nc.vector.tensor_copy(TL[:, :], TD[:, :])
            pen = cpool.tile([32, 128], f32, tag=f"pen{li}")
            nc.vector.tensor_scalar(pen[:, :], TL[:, :], 0.5, 1e6,
                                    Alu.is_lt, Alu.mult)
            cmin = cpool.tile([32, 128], f32, tag=f"cmin{li}")
            nc.gpsimd.tensor_tensor(cmin[:, :], pen[:, :], iota[:, :], Alu.add)
            cmax = cpool.tile([32, 128], f32, tag=f"cmax{li}")
            nc.gpsimd.tensor_tensor(cmax[:, :], iota[:, :], pen[:, :],
                                    Alu.subtract)
            Y = cpool.tile([32, 2], f32, tag=f"Y{li}")
            nc.vector.tensor_reduce(Y[:, 0:1], cmin[:, :], X_AX, op=Alu.min)
            nc.vector.tensor_reduce(Y[:, 1:2], cmax[:, :], X_AX, op=Alu.max)
            YTmin = ppool.tile([1, 32], f32, tag=f"YTmin{li}")
            YTmax = ppool.tile([1, 32], f32, tag=f"YTmax{li}")
            nc.tensor.transpose(YTmin[:, :], Y[:, 0:1], ident[0:32, 0:32])
            nc.tensor.transpose(YTmax[:, :], Y[:, 1:2], ident[0:32, 0:32])
            ymin8 = cpool.tile([1, 8], f32, tag=f"ymin{li}")
            ymax8 = cpool.tile([1, 8], f32, tag=f"ymax{li}")
            nc.vector.tensor_reduce(
                ymin8[:, :], YTmin[0:1, :].rearrange("p (b s) -> p s b", b=4),
                X_AX, op=Alu.min)
            nc.vector.tensor_reduce(
                ymax8[:, :], YTmax[0:1, :].rearrange("p (b s) -> p s b", b=4),
                X_AX, op=Alu.max)
            hL = cpool.tile([1, 8], f32, tag=f"h{li}")
            nc.vector.tensor_tensor(hL[:, :], ymax8[:, :], ymin8[:, :],
                                    Alu.subtract)
            nc.vector.tensor_scalar_max(hL[:, :], hL[:, :], 0.0)
            heights.append(hL)

        h_cup, h_disc = heights
        den = cpool.tile([1, 8], f32, tag="den")
        nc.vector.tensor_scalar_add(den[:, :], h_disc[:, :], 1e-6)
        rec = cpool.tile([1, 8], f32, tag="rec")
        nc.vector.reciprocal(rec[:, :], den[:, :])
        nc.vector.tensor_tensor(O[:, 0:8], h_cup[:, :], rec[:, :], Alu.mult)

        ms1 = cpool.tile([1, 8], f32, tag="ms1")
        ms2 = cpool.tile([1, 8], f32, tag="ms2")
        nc.vector.tensor_reduce(
            ms1[:, :], S12[0:1, 0:32].rearrange("p (b s) -> p s b", b=4),
            X_AX, op=Alu.add)
        nc.vector.tensor_reduce(
            ms2[:, :], S12[0:1, 32:64].rearrange("p (b s) -> p s b", b=4),
            X_AX, op=Alu.add)
        sc = 1.0 / HW
        nc.vector.tensor_scalar_mul(O[:, 8:16], ms2[:, :], sc)
        nc.vector.tensor_scalar_mul(O[:, 16:24], ms1[:, :], sc)
        nc.vector.tensor_scalar_mul(O[:, 24:32], ms2[:, :], sc)
        nc.vector.tensor_scalar_mul(O[:, 32:40], ms1[:, :], sc)

        nc.sync.dma_start(out[:, :], O[:, :])

    nc.finalize()
    return nc


def _get_nc():
    if "nc" not in _CACHE:
        _CACHE["nc"] = _build()
    return _CACHE["nc"]


def _host_inputs():
    iota = (np.arange(128, dtype=np.float32)[None, :]
            + 128.0 * np.repeat(np.arange(4, dtype=np.float32), 8)[:, None])
    ident = np.eye(128, dtype=np.float32)
    ones = np.ones((128, 1), dtype=np.float32)
    return iota, ident, ones


def _run(seg_mask, trace=False):
    from concourse.bass_utils import run_bass_kernel_spmd

    x = np.ascontiguousarray(np.asarray(seg_mask, dtype=np.float32))
    assert x.shape == (B, C, H, W)
    iota, ident, ones = _host_inputs()
    in_maps = [
        {"x": x[SPC * c:SPC * (c + 1)], "iota": iota, "ident": ident,
         "ones": ones}
        for c in range(NCORES)
    ]
    nc = _get_nc()
    res = run_bass_kernel_spmd(nc, in_maps, core_ids=list(range(NCORES)),
                               trace=trace)
    outs = []
    for c in range(NCORES):
        o = np.asarray(res.results[c]["out"]).reshape(5, SPC).T
        outs.append(o)
    full = np.concatenate(outs, axis=0).astype(np.float32)
    return full, res


def kernel(segmentation_mask):
    full, _ = _run(segmentation_mask, trace=False)
    return full



# revision 29
# speedup vs baseline: 1.2741x; 1.2741x over previous
"""Trainium2 Bass kernel for CDRExtractor (segment_reduce).

Input : segmentation_mask (64, 3, 512, 512) fp32
Output: (64, 5) fp32 = [cdr, disc_mean, cup_mean, disc_mean, cup_mean]

Sharding: pure data parallel, 8 samples per core across 8 cores; each core
streams its 24 MiB shard once.

v2 design (vs the 95.0us baseline):
  - The input DMA (75.8us of transfer cost at the modeled per-queue rate)
    is split across THREE DGE queues (SP + ACT + Pool) so it overlaps with
    itself; SP carries ~80%, the compute engines absorb the rest in their
    slack.
  - d-tests (argmax row-presence) run in t-space: MM = max(t_other, 0) via
    DVE tensor_scalar (4x uop), count via DVE STT is_gt w/ fused accum.
    No exp needed on the d-path.
  - softmax sums: p = f*r products on Pool (3/4) + DVE (1/4), then plane
    sums via PE one-hot matmuls accumulated in two PSUM banks [8, 512]
    (PE is idle otherwise), final [8,512]->[8,1] reduces at the tail.
  - ACT does exp(T) [128,4096], ln(1+sadd) [128,2048], exp(-L) [128,2048]
    per 4-sample tile - fat instructions amortize the 185ns ACT init.
  - Tapered schedule: 1/1/2-sample tiles at the start (fast pipeline fill)
    and 2/1/1 at the end (short drain chain).

Engine busy targets per core (v1 CoreSim cost model): ACT ~61us (compute
59 + dma), Pool ~61us (subs+products+dma), DVE ~61us (sadd+MM+STT+some
products), SP ~61us dma, PE ~14-50us (matmuls; p-state dependent).
"""

import numpy as np
from contextlib import ExitStack

B, C, H, W = 64, 3, 512, 512
NCORES = 8
SPC = B // NCORES      # samples per core = 8
NB = H // 128          # 128-row blocks = 4
HW = float(H * W)

_CACHE = {}

# schedule: (s0, ns, b) tiles; tapered at both ends
TILES = [
    (0, 2, 0), (2, 2, 0), (4, 4, 0),
    (0, 4, 1), (4, 4, 1),
    (0, 4, 2), (4, 4, 2),
    (0, 4, 3), (4, 2, 3), (6, 2, 3),
]
P2_DVE_NS = 0      # leading samples of p2 on DVE for fat tiles
EXTRA_UNIT = "pool"  # who gets the 12th dma unit on even fat tiles
SPLIT_EXP = False   # exp in 2-sample halves
SPLIT_SUB = True   # subs in 2-sample halves
END_RR = False      # last two tiles round-robin their dma chunks
FILL_RR = 2         # tiles with index < FILL_RR round-robin their dma chunks
MS1_ON_ACT = False  # (unused)
POOL_UNITS = 3      # pool dma units per fat tile (excl. EXTRA)
SADD_POOL_NS = 1    # trailing samples of sadd on Pool for fat tiles
FILL_ACT = True     # include ACT in fill-phase dma round-robin
MS2_DVE = False     # PS2 reduce on DVE instead of ACT copy-accum
RECIP_TILES = (4, 6)    # tiles whose r comes from DVE reciprocal (ACT relief)
D_MPATH = True     # d-counts via margin mins + 4x ts-accum


def _build():
    import concourse.bass as bass
    import concourse.bacc as bacc
    import concourse.mybir as mybir
    from concourse.tile import TileContext

    # Offer only the act-table set containing BOTH exp and ln so the act
    # table never reloads mid-kernel.
    if not _CACHE.get("act_patch"):
        _orig_tables = bacc.get_activation_tables

        def _only_ln_exp(arch):
            t = _orig_tables(arch)
            keep = "natural_log_exp_and_others"
            return {k: (v if k == keep else set()) for k, v in t.items()}

        bacc.get_activation_tables = _only_ln_exp
        _CACHE["act_patch"] = True

    f32 = mybir.dt.float32
    bf16 = mybir.dt.bfloat16
    Alu = mybir.AluOpType
    AFT = mybir.ActivationFunctionType
    X_AX = mybir.AxisListType.X

    nc = bacc.Bacc()
    x = nc.dram_tensor("x", (SPC, C, H, W), f32, kind="ExternalInput")
    iota_in = nc.dram_tensor("iota", (32, 128), f32, kind="ExternalInput")
    ident_in = nc.dram_tensor("ident", (128, 128), f32, kind="ExternalInput")
    oh_in = nc.dram_tensor("oh", (128, 64), bf16, kind="ExternalInput")
    out = nc.dram_tensor("out", (5, SPC), f32, kind="ExternalOutput")

    with TileContext(nc) as tc, ExitStack() as ctx:
        cpool = ctx.enter_context(tc.tile_pool(name="consts", bufs=1))
        apool = ctx.enter_context(tc.tile_pool(name="accs", bufs=1))
        mpool = ctx.enter_context(tc.tile_pool(name="main", bufs=2))
        ppool = ctx.enter_context(tc.tile_pool(name="ps", bufs=1, space="PSUM"))

        # dummy activation on a memset tile: forces the (one-time) act
        # table load to run at t~0 instead of behind the first X DMA
        warm = cpool.tile([1, 16], bf16, tag="warm")
        nc.vector.memset(warm[:, :], 0.0)
        nc.scalar.activation(warm[:, :], warm[:, :], AFT.Exp)

        iota = cpool.tile([32, 128], f32, tag="iota")
        ident = cpool.tile([128, 128], f32, tag="ident")
        oh = cpool.tile([128, 64], bf16, tag="oh")
        # one-hot stationaries needed from the first matmul on
        nc.gpsimd.dma_start(oh[:, :], oh_in[:, :])

        # accumulators: col j = b*8 + s
        DM1 = apool.tile([128, 32], f32, tag="DM1")  # row argmax counts lbl1
        DM2 = apool.tile([128, 32], f32, tag="DM2")
        # psum accumulators for plane sums: row = sample
        PS0 = ppool.tile([8, 512], f32, tag="PS0")
        PS2 = ppool.tile([8, 512], f32, tag="PS2")
        DVE_P2 = {4, 6}
        mm_count = [0]
        MM_TOTAL = sum(ns for (_, ns, _) in TILES) * 2

        # per-tile DMA chunk assignment. SP carries ~80% of the bytes;
        # Pool absorbs ~2 units/fat tile; ACT only helps during the fill
        # (it is the busiest engine otherwise).
        def dma_chunks(i, s0, ns, b):
            units = [(s, c) for s in range(s0, s0 + ns) for c in range(C)]
            if i < FILL_RR or (END_RR and i >= len(TILES) - 2):
                # fill/drain phase: all queues in parallel
                order = ["sp", "act", "pool"] if FILL_ACT else ["sp", "pool"]
                return [(order[j % len(order)], s, c)
                        for j, (s, c) in enumerate(units)]
            if ns == 4:
                last = EXTRA_UNIT if i % 2 == 0 else "sp"
                npool = POOL_UNITS
                nsp = 11 - npool
                qs = []
                for j in range(11):
                    if npool and j % (11 // npool + 1) == (11 // npool):
                        qs.append("pool")
                        npool -= 1
                    else:
                        qs.append("sp")
                qs.append(last)
            elif ns == 2:
                qs = ["sp"] * 4 + ["pool", "sp"]
            else:
                qs = ["sp"] * 3
            return [(q, s, c) for (s, c), q in zip(units, qs)]

        def stage_dma(i):
            s0, ns, b = TILES[i]
            Xf = mpool.tile([128, 4 * C * W], f32, tag="X",
                            name=f"X_{i}", bufs=4)
            X = Xf[:, 0:ns * C * W]
            Xv = X.rearrange("p (s c w) -> p s c w", s=ns, c=C)
            for q, s, c in dma_chunks(i, s0, ns, b):
                src = x[s, c, b * 128:(b + 1) * 128, :]
                dst = Xv[:, s - s0, c, :]
                if q == "sp":
                    nc.sync.dma_start(dst, src)
                elif q == "act":
                    nc.scalar.dma_start(dst, src)
                else:
                    nc.gpsimd.dma_start(dst, src)
            return X

        def stage_sub(i, X):
            """Pool: T = [x1-x0 | x2-x0] in bf16, laid out (s, l, w)."""
            s0, ns, b = TILES[i]
            Xv = X.rearrange("p (s c w) -> p s c w", s=ns, c=C)
            Tf = mpool.tile([128, 4 * 2 * W], bf16, tag="T",
                            name=f"T_{i}", bufs=3)
            T = Tf[:, 0:ns * 2 * W]
            Tv = T.rearrange("p (s l w) -> p s l w", s=ns, l=2)
            if SPLIT_SUB and ns == 4:
                for h in (slice(0, 2), slice(2, 4)):
                    for li in range(2):
                        nc.gpsimd.tensor_tensor(
                            Tv[:, h, li, :], Xv[:, h, li + 1, :],
                            Xv[:, h, 0, :], Alu.subtract)
            else:
                for li in range(2):
                    nc.gpsimd.tensor_tensor(
                        Tv[:, :, li, :], Xv[:, :, li + 1, :], Xv[:, :, 0, :],
                        Alu.subtract)
            return T

        def stage_exp(i, T):
            """ACT: F = exp(T) bf16."""
            s0, ns, b = TILES[i]
            Ff = mpool.tile([128, 4 * 2 * W], bf16, tag="F",
                            name=f"F_{i}", bufs=3)
            F = Ff[:, 0:ns * 2 * W]
            if SPLIT_EXP and ns == 4:
                h = ns * W
                nc.scalar.activation(F[:, 0:h], T[:, 0:h], AFT.Exp)
                nc.scalar.activation(F[:, h:2 * h], T[:, h:2 * h], AFT.Exp)
            else:
                nc.scalar.activation(F[:, :], T[:, :], AFT.Exp)
            return F

        def stage_sadd(i, F):
            """DVE: sadd = f1 + f2 (bf16, 2x uop)."""
            s0, ns, b = TILES[i]
            Fv = F.rearrange("p (s l w) -> p s l w", s=ns, l=2)
            sf = mpool.tile([128, 4 * W], bf16, tag="sa",
                            name=f"sa_{i}", bufs=3)
            sadd = sf[:, 0:ns * W]
            sv = sadd.rearrange("p (s w) -> p s w", s=ns)
            k = ns - SADD_POOL_NS if ns == 4 else ns
            if k > 0:
                nc.vector.tensor_tensor(sv[:, 0:k, :], Fv[:, 0:k, 0, :],
                                        Fv[:, 0:k, 1, :], Alu.add)
            if k < ns:
                nc.gpsimd.tensor_tensor(sv[:, k:ns, :], Fv[:, k:ns, 0, :],
                                        Fv[:, k:ns, 1, :], Alu.add)
            return sadd

        def stage_mm(i, T):
            """DVE: either MM = max(t_other, 0) (4x), or margin mins
            m_l = min(t_l, t_l - t_other) for the m-path."""
            s0, ns, b = TILES[i]
            Tv = T.rearrange("p (s l w) -> p s l w", s=ns, l=2)
            MMf = mpool.tile([128, 4 * 2 * W], bf16, tag="MM",
                             name=f"MM_{i}", bufs=1)
            MM = MMf[:, 0:ns * 2 * W]
            MMv = MM.rearrange("p (l s w) -> p l s w", l=2, s=ns)
            if D_MPATH:
                UVf = mpool.tile([128, 4 * 2 * W], bf16, tag="UV",
                                 name=f"UV_{i}", bufs=1)
                u = UVf[:, 0:ns * W]
                v = UVf[:, 4 * W:4 * W + ns * W]
                uv = u.rearrange("p (s w) -> p s w", s=ns)
                nc.vector.tensor_tensor(uv[:, :, :], Tv[:, :, 0, :],
                                        Tv[:, :, 1, :], Alu.subtract)
                nc.vector.tensor_scalar_mul(v[:, :], u[:, :], -1.0)
                vv = v.rearrange("p (s w) -> p s w", s=ns)
                nc.vector.tensor_tensor(MMv[:, 0, :, :], Tv[:, :, 0, :],
                                        uv[:, :, :], Alu.min)
                nc.vector.tensor_tensor(MMv[:, 1, :, :], Tv[:, :, 1, :],
                                        vv[:, :, :], Alu.min)
            else:
                nc.vector.tensor_scalar(MMv[:, 0, :, :], Tv[:, :, 1, :], 0.0,
                                        None, Alu.max)
                nc.vector.tensor_scalar(MMv[:, 1, :, :], Tv[:, :, 0, :], 0.0,
                                        None, Alu.max)
            return MM

        def stage_d(i, T, MM):
            """DVE: d-counts. MM-path: STT is_gt w/ accum (1x).
            m-path: tensor_scalar is_gt/add w/ accum (4x)."""
            s0, ns, b = TILES[i]
            Tv = T.rearrange("p (s l w) -> p s l w", s=ns, l=2)
            MMv = MM.rearrange("p (l s w) -> p l s w", l=2, s=ns)
            df = mpool.tile([128, 4 * 2 * W], bf16, tag="ds",
                            name=f"ds_{i}", bufs=1)
            dscr = df[:, 0:ns * 2 * W]
            dv = dscr.rearrange("p (s l w) -> p s l w", s=ns, l=2)
            for si in range(ns):
                col = b * 8 + (s0 + si)
                for li, DM in ((0, DM1), (1, DM2)):
                    if D_MPATH:
                        nc.vector.tensor_scalar(
                            dv[:, si, li, :], MMv[:, li, si, :], 0.0, 0.0,
                            Alu.is_gt, Alu.add,
                            accum_out=DM[:, col:col + 1])
                    else:
                        nc.vector.scalar_tensor_tensor(
                            dv[:, si, li, :], Tv[:, si, li, :], 0.0,
                            MMv[:, li, si, :], Alu.add, Alu.is_gt,
                            accum_out=DM[:, col:col + 1])

        def stage_lnrexp(i, sadd):
            """r = 1/(1+f1+f2): ACT ln+exp, or DVE recip for RECIP_TILES."""
            s0, ns, b = TILES[i]
            lf = mpool.tile([128, 4 * W], bf16, tag="ln",
                            name=f"ln_{i}", bufs=2)
            lns = lf[:, 0:ns * W]
            rf = mpool.tile([128, 4 * W], bf16, tag="rb",
                            name=f"rb_{i}", bufs=2)
            rb = rf[:, 0:ns * W]
            if i in RECIP_TILES:
                nc.gpsimd.tensor_scalar_add(lns[:, :], sadd[:, :], 1.0)
                with nc.allow_low_precision(reason="bf16 softmax r"):
                    nc.vector.reciprocal(rb[:, :], lns[:, :])
            else:
                nc.scalar.activation(lns[:, :], sadd[:, :], AFT.Ln, bias=1.0)
                nc.scalar.activation(rb[:, :], lns[:, :], AFT.Exp, scale=-1.0)
            return rb

        def stage_prod(i, F, rb):
            """p2 = f2 * r only; p1-sums come from N - sum(r) - sum(p2)."""
            s0, ns, b = TILES[i]
            Fv = F.rearrange("p (s l w) -> p s l w", s=ns, l=2)
            rv = rb.rearrange("p (s w) -> p s w", s=ns)
            pf = mpool.tile([128, 4 * W], bf16, tag="pc",
                            name=f"pc_{i}", bufs=1)
            pscr = pf[:, 0:ns * W]
            pv = pscr.rearrange("p (s w) -> p s w", s=ns)
            if ns == 4:
                k = P2_DVE_NS
                if k > 0:
                    nc.vector.tensor_tensor(pv[:, 0:k, :],
                                            Fv[:, 0:k, 1, :],
                                            rv[:, 0:k, :], Alu.mult)
                nc.gpsimd.tensor_tensor(pv[:, k:4, :], Fv[:, k:4, 1, :],
                                        rv[:, k:4, :], Alu.mult)
            else:
                nc.vector.tensor_tensor(pv[:, :, :], Fv[:, :, 1, :],
                                        rv[:, :, :], Alu.mult)
            return pscr

        def stage_pe(i, rb, pscr):
            """PE: one-hot matmuls accumulate plane sums into PSUM.
            PS0 accumulates r (= p0), PS2 accumulates p2."""
            s0, ns, b = TILES[i]
            pv = pscr.rearrange("p (s w) -> p s w", s=ns)
            rv = rb.rearrange("p (s w) -> p s w", s=ns)
            for si in range(ns):
                s = s0 + si
                for src_v, PS in ((rv, PS0), (pv, PS2)):
                    k = mm_count[0]
                    mm_count[0] += 1
                    nc.tensor.matmul(PS[:, :], oh[:, 8 * s:8 * s + 8],
                                     src_v[:, si, :],
                                     start=(k < 2), stop=(k >= MM_TOTAL - 2))

        # ---- software-pipelined emission (2-tile skew) ----
        # ACT order per iteration: [ln/rexp for i-2, exp for i] so the
        # ln of tile i runs two iterations later than its sadd (no ACT
        # stall on the DVE round trip); products likewise at i-2.
        N = len(TILES)
        Xs = {}
        state = {}   # i -> (F, sadd)

        def drain_stage(j):
            Fm, saddm = state.pop(j)
            rbm = stage_lnrexp(j, saddm)
            pscr = stage_prod(j, Fm, rbm)
            stage_pe(j, rbm, pscr)

        for i in range(N):
            if i == 0:
                Xs[0] = stage_dma(0)
                Xs[1] = stage_dma(1)
            T = stage_sub(i, Xs[i])
            if i >= 2:
                drain_stage(i - 2)
            F = stage_exp(i, T)
            MM = stage_mm(i, T)
            sadd = stage_sadd(i, F)
            stage_d(i, T, MM)
            if i + 2 < N:
                Xs[i + 2] = stage_dma(i + 2)
            state[i] = (F, sadd)
        drain_stage(N - 2)
        drain_stage(N - 1)

        # ---- tail ----
        # const loads for the d-tail (deferred; they aren't needed earlier)
        nc.sync.dma_start(iota[:, :], iota_in[:, :])
        nc.sync.dma_start(ident[:, :], ident_in[:, :])
        O = cpool.tile([1, 40], f32, tag="O")

        # p-tail: PSUM [8,512] -> [8,1] sums (DVE + ACT in parallel), then
        # transpose to [1,8]
        ms0 = cpool.tile([8, 1], f32, tag="ms0")
        ms2 = cpool.tile([8, 1], f32, tag="ms2")
        msum = cpool.tile([8, 1], f32, tag="msum")
        ms2scr = cpool.tile([8, 512], f32, tag="ms2scr")
        nc.vector.tensor_reduce(ms0[:, :], PS0[:, :], X_AX, op=Alu.add)
        if MS2_DVE:
            nc.vector.tensor_reduce(ms2[:, :], PS2[:, :], X_AX, op=Alu.add)
        else:
            nc.scalar.activation(ms2scr[:, :], PS2[:, :], AFT.Copy,
                                 accum_out=ms2[:, :])
        nc.vector.tensor_tensor(msum[:, :], ms0[:, :], ms2[:, :], Alu.add)
        MT = ppool.tile([1, 16], f32, tag="MT")
        nc.tensor.transpose(MT[:, 0:8], msum[:, :], ident[0:8, 0:8])
        nc.tensor.transpose(MT[:, 8:16], ms2[:, :], ident[0:8, 0:8])
        sc = 1.0 / HW
        # rows: [cdr, disc=l2, cup=l1, disc, cup]; cup = 1 - (p0m + p2m)
        nc.vector.tensor_scalar(O[:, 8:16], MT[:, 8:16], sc, None, Alu.mult)
        nc.vector.tensor_scalar(O[:, 16:24], MT[:, 0:8], -sc, 1.0,
                                Alu.mult, Alu.add)
        nc.vector.tensor_scalar(O[:, 24:32], MT[:, 8:16], sc, None, Alu.mult)
        nc.vector.tensor_scalar(O[:, 32:40], MT[:, 0:8], -sc, 1.0,
                                Alu.mult, Alu.add)

        # d-tail: heights from DM1/DM2 (as in baseline)
        heights = []
        dparts = []
        for li, DM in enumerate((DM1, DM2)):
            TD = ppool.tile([32, 128], f32, tag="TD", name=f"TD{li}")
            nc.tensor.transpose(TD[:, :], DM[:, :], ident[:, :])
            pen = cpool.tile([32, 128], f32, tag=f"pen{li}")
            nc.vector.tensor_scalar(pen[:, :], TD[:, :], 0.5, 1e6,
                                    Alu.is_lt, Alu.mult)
            cmin = cpool.tile([32, 128], f32, tag=f"cmin{li}")
            nc.gpsimd.tensor_tensor(cmin[:, :], pen[:, :], iota[:, :], Alu.add)
            cmax = cpool.tile([32, 128], f32, tag=f"cmax{li}")
            nc.gpsimd.tensor_tensor(cmax[:, :], iota[:, :], pen[:, :],
                                    Alu.subtract)
            Y = cpool.tile([32, 2], f32, tag=f"Y{li}")
            nc.vector.tensor_reduce(Y[:, 0:1], cmin[:, :], X_AX, op=Alu.min)
            nc.vector.tensor_reduce(Y[:, 1:2], cmax[:, :], X_AX, op=Alu.max)
            YTmin = ppool.tile([1, 32], f32, tag="YTmin", name=f"YTmin{li}")
            YTmax = ppool.tile([1, 32], f32, tag="YTmax", name=f"YTmax{li}")
            nc.tensor.transpose(YTmin[:, :], Y[:, 0:1], ident[0:32, 0:32])
            nc.tensor.transpose(YTmax[:, :], Y[:, 1:2], ident[0:32, 0:32])
            ymin8 = cpool.tile([1, 8], f32, tag=f"ymin{li}")
            ymax8 = cpool.tile([1, 8], f32, tag=f"ymax{li}")
            nc.vector.tensor_reduce(
                ymin8[:, :], YTmin[0:1, :].rearrange("p (b s) -> p s b", b=4),
                X_AX, op=Alu.min)
            nc.vector.tensor_reduce(
                ymax8[:, :], YTmax[0:1, :].rearrange("p (b s) -> p s b", b=4),
                X_AX, op=Alu.max)
            hL = cpool.tile([1, 8], f32, tag=f"h{li}")
            nc.vector.tensor_tensor(hL[:, :], ymax8[:, :], ymin8[:, :],
                                    Alu.subtract)
            nc.vector.tensor_scalar_max(hL[:, :], hL[:, :], 0.0)
            heights.append(hL)

        h_cup, h_disc = heights
        den = cpool.tile([1, 8], f32, tag="den")
        nc.vector.tensor_scalar_add(den[:, :], h_disc[:, :], 1e-6)
        rec = cpool.tile([1, 8], f32, tag="rec")
        nc.vector.reciprocal(rec[:, :], den[:, :])
        nc.vector.tensor_tensor(O[:, 0:8], h_cup[:, :], rec[:, :], Alu.mult)

        nc.sync.dma_start(out[:, :], O[:, :])

    nc.finalize()
    return nc


def _get_nc():
    if "nc" not in _CACHE:
        _CACHE["nc"] = _build()
    return _CACHE["nc"]


def _host_inputs():
    iota = (np.arange(128, dtype=np.float32)[None, :]
            + 128.0 * np.repeat(np.arange(4, dtype=np.float32), 8)[:, None])
    ident = np.eye(128, dtype=np.float32)
    # oh[:, 8s+j] = 1 iff j == s (one-hot stationary for per-sample matmul)
    import ml_dtypes
    oh = np.zeros((128, 64), dtype=ml_dtypes.bfloat16)
    for s in range(8):
        oh[:, 8 * s + s] = 1.0
    return iota, ident, oh


def _run(seg_mask, trace=False):
    from concourse.bass_utils import run_bass_kernel_spmd

    x = np.ascontiguousarray(np.asarray(seg_mask, dtype=np.float32))
    assert x.shape == (B, C, H, W)
    iota, ident, oh = _host_inputs()
    in_maps = [
        {"x": x[SPC * c:SPC * (c + 1)], "iota": iota, "ident": ident,
         "oh": oh}
        for c in range(NCORES)
    ]
    nc = _get_nc()
    res = run_bass_kernel_spmd(nc, in_maps, core_ids=list(range(NCORES)),
                               trace=trace)
    outs = []
    for c in range(NCORES):
        o = np.asarray(res.results[c]["out"]).reshape(5, SPC).T
        outs.append(o)
    full = np.concatenate(outs, axis=0).astype(np.float32)
    return full, res


def kernel(segmentation_mask):
    full, _ = _run(segmentation_mask, trace=False)
    return full


# revision 40
# speedup vs baseline: 1.2922x; 1.0142x over previous
"""Trainium2 Bass kernel for CDRExtractor (segment_reduce).

Input : segmentation_mask (64, 3, 512, 512) fp32
Output: (64, 5) fp32 = [cdr, disc_mean, cup_mean, disc_mean, cup_mean]

Sharding: pure data parallel, 8 samples per core across 8 cores; each core
streams its 24 MiB shard once.

v2 design (vs the 95.0us baseline):
  - The input DMA (75.8us of transfer cost at the modeled per-queue rate)
    is split across THREE DGE queues (SP + ACT + Pool) so it overlaps with
    itself; SP carries ~80%, the compute engines absorb the rest in their
    slack.
  - d-tests (argmax row-presence) run in t-space: MM = max(t_other, 0) via
    DVE tensor_scalar (4x uop), count via DVE STT is_gt w/ fused accum.
    No exp needed on the d-path.
  - softmax sums: p = f*r products on Pool (3/4) + DVE (1/4), then plane
    sums via PE one-hot matmuls accumulated in two PSUM banks [8, 512]
    (PE is idle otherwise), final [8,512]->[8,1] reduces at the tail.
  - ACT does exp(T) [128,4096], ln(1+sadd) [128,2048], exp(-L) [128,2048]
    per 4-sample tile - fat instructions amortize the 185ns ACT init.
  - Tapered schedule: 1/1/2-sample tiles at the start (fast pipeline fill)
    and 2/1/1 at the end (short drain chain).

Engine busy targets per core (v1 CoreSim cost model): ACT ~61us (compute
59 + dma), Pool ~61us (subs+products+dma), DVE ~61us (sadd+MM+STT+some
products), SP ~61us dma, PE ~14-50us (matmuls; p-state dependent).
"""

import numpy as np
from contextlib import ExitStack

B, C, H, W = 64, 3, 512, 512
NCORES = 8
SPC = B // NCORES      # samples per core = 8
NB = H // 128          # 128-row blocks = 4
HW = float(H * W)

_CACHE = {}

# schedule: (s0, ns, b) tiles; tapered at both ends
TILES = [
    (0, 2, 0), (2, 2, 0), (4, 4, 0),
    (0, 4, 1), (4, 4, 1),
    (0, 4, 2), (4, 4, 2),
    (0, 4, 3), (4, 2, 3), (6, 2, 3),
]
P2_DVE_NS = 0      # leading samples of p2 on DVE for fat tiles
EXTRA_UNIT = "act"  # who gets the 12th dma unit on even fat tiles
SPLIT_EXP = False   # exp in 2-sample halves
SPLIT_SUB = True   # subs in 2-sample halves
END_RR = False      # last two tiles round-robin their dma chunks
FILL_RR = 3         # tiles with index < FILL_RR round-robin their dma chunks
MS1_ON_ACT = False  # (unused)
POOL_UNITS = 3      # pool dma units per fat tile (excl. EXTRA)
SADD_POOL_NS = 1    # trailing samples of sadd on Pool for fat tiles
FILL_ACT = True     # include ACT in fill-phase dma round-robin
MS2_DVE = False     # PS2 reduce on DVE instead of ACT copy-accum
RECIP_TILES = (4, 6)    # tiles whose r comes from DVE reciprocal (ACT relief)
D_MPATH = True     # d-counts via margin mins + 4x ts-accum
SPLIT_SUB2 = True  # ns=2 subs split per sample
MPATH_SPLIT_FIRST = 0  # tiles < this get per-sample m-path ops (fill)
T_BUFS = 3
F_BUFS = 3
RB_BUFS = 2
SA_BUFS = 3


def _build():
    import concourse.bass as bass
    import concourse.bacc as bacc
    import concourse.mybir as mybir
    from concourse.tile import TileContext

    # Offer only the act-table set containing BOTH exp and ln so the act
    # table never reloads mid-kernel.
    if not _CACHE.get("act_patch"):
        _orig_tables = bacc.get_activation_tables

        def _only_ln_exp(arch):
            t = _orig_tables(arch)
            keep = "natural_log_exp_and_others"
            return {k: (v if k == keep else set()) for k, v in t.items()}

        bacc.get_activation_tables = _only_ln_exp
        _CACHE["act_patch"] = True

    f32 = mybir.dt.float32
    bf16 = mybir.dt.bfloat16
    Alu = mybir.AluOpType
    AFT = mybir.ActivationFunctionType
    X_AX = mybir.AxisListType.X

    nc = bacc.Bacc()
    x = nc.dram_tensor("x", (SPC, C, H, W), f32, kind="ExternalInput")
    iota_in = nc.dram_tensor("iota", (32, 128), f32, kind="ExternalInput")
    ident_in = nc.dram_tensor("ident", (128, 128), f32, kind="ExternalInput")
    oh_in = nc.dram_tensor("oh", (128, 64), bf16, kind="ExternalInput")
    out = nc.dram_tensor("out", (5, SPC), f32, kind="ExternalOutput")

    with TileContext(nc) as tc, ExitStack() as ctx:
        cpool = ctx.enter_context(tc.tile_pool(name="consts", bufs=1))
        apool = ctx.enter_context(tc.tile_pool(name="accs", bufs=1))
        mpool = ctx.enter_context(tc.tile_pool(name="main", bufs=2))
        ppool = ctx.enter_context(tc.tile_pool(name="ps", bufs=1, space="PSUM"))

        # dummy activation on a memset tile: forces the (one-time) act
        # table load to run at t~0 instead of behind the first X DMA
        warm = cpool.tile([1, 16], bf16, tag="warm")
        nc.vector.memset(warm[:, :], 0.0)
        nc.scalar.activation(warm[:, :], warm[:, :], AFT.Exp)

        iota = cpool.tile([32, 128], f32, tag="iota")
        ident = cpool.tile([128, 128], f32, tag="ident")
        oh = cpool.tile([128, 64], bf16, tag="oh")
        # one-hot stationaries needed from the first matmul on
        nc.gpsimd.dma_start(oh[:, :], oh_in[:, :])

        # accumulators: col j = b*8 + s
        DM1 = apool.tile([128, 32], f32, tag="DM1")  # row argmax counts lbl1
        DM2 = apool.tile([128, 32], f32, tag="DM2")
        # psum accumulators for plane sums: row = sample
        PS0 = ppool.tile([8, 512], f32, tag="PS0")
        PS2 = ppool.tile([8, 512], f32, tag="PS2")
        DVE_P2 = {4, 6}
        mm_count = [0]
        MM_TOTAL = sum(ns for (_, ns, _) in TILES) * 2

        # per-tile DMA chunk assignment. SP carries ~80% of the bytes;
        # Pool absorbs ~2 units/fat tile; ACT only helps during the fill
        # (it is the busiest engine otherwise).
        def dma_chunks(i, s0, ns, b):
            units = [(s, c) for s in range(s0, s0 + ns) for c in range(C)]
            if i < FILL_RR or (END_RR and i >= len(TILES) - 2):
                # fill/drain phase: all queues in parallel
                order = ["sp", "act", "pool"] if FILL_ACT else ["sp", "pool"]
                return [(order[j % len(order)], s, c)
                        for j, (s, c) in enumerate(units)]
            if ns == 4:
                last = EXTRA_UNIT if i % 2 == 0 else "sp"
                npool = POOL_UNITS
                nsp = 11 - npool
                qs = []
                for j in range(11):
                    if npool and j % (11 // npool + 1) == (11 // npool):
                        qs.append("pool")
                        npool -= 1
                    else:
                        qs.append("sp")
                qs.append(last)
            elif ns == 2:
                qs = ["sp"] * 4 + ["pool", "sp"]
            else:
                qs = ["sp"] * 3
            return [(q, s, c) for (s, c), q in zip(units, qs)]

        def stage_dma(i):
            s0, ns, b = TILES[i]
            Xf = mpool.tile([128, 4 * C * W], f32, tag="X",
                            name=f"X_{i}", bufs=4)
            X = Xf[:, 0:ns * C * W]
            Xv = X.rearrange("p (s c w) -> p s c w", s=ns, c=C)
            for q, s, c in dma_chunks(i, s0, ns, b):
                src = x[s, c, b * 128:(b + 1) * 128, :]
                dst = Xv[:, s - s0, c, :]
                if q == "sp":
                    nc.sync.dma_start(dst, src)
                elif q == "act":
                    nc.scalar.dma_start(dst, src)
                else:
                    nc.gpsimd.dma_start(dst, src)
            return X

        def stage_sub(i, X):
            """Pool: T = [x1-x0 | x2-x0] in bf16, laid out (s, l, w)."""
            s0, ns, b = TILES[i]
            Xv = X.rearrange("p (s c w) -> p s c w", s=ns, c=C)
            Tf = mpool.tile([128, 4 * 2 * W], bf16, tag="T",
                            name=f"T_{i}", bufs=T_BUFS)
            T = Tf[:, 0:ns * 2 * W]
            Tv = T.rearrange("p (s l w) -> p s l w", s=ns, l=2)
            if SPLIT_SUB and ns == 4:
                for h in (slice(0, 2), slice(2, 4)):
                    for li in range(2):
                        nc.gpsimd.tensor_tensor(
                            Tv[:, h, li, :], Xv[:, h, li + 1, :],
                            Xv[:, h, 0, :], Alu.subtract)
            elif SPLIT_SUB2 and ns == 2:
                for h in (slice(0, 1), slice(1, 2)):
                    for li in range(2):
                        nc.gpsimd.tensor_tensor(
                            Tv[:, h, li, :], Xv[:, h, li + 1, :],
                            Xv[:, h, 0, :], Alu.subtract)
            else:
                for li in range(2):
                    nc.gpsimd.tensor_tensor(
                        Tv[:, :, li, :], Xv[:, :, li + 1, :], Xv[:, :, 0, :],
                        Alu.subtract)
            return T

        def stage_exp(i, T):
            """ACT: F = exp(T) bf16."""
            s0, ns, b = TILES[i]
            Ff = mpool.tile([128, 4 * 2 * W], bf16, tag="F",
                            name=f"F_{i}", bufs=F_BUFS)
            F = Ff[:, 0:ns * 2 * W]
            if SPLIT_EXP and ns == 4:
                h = ns * W
                nc.scalar.activation(F[:, 0:h], T[:, 0:h], AFT.Exp)
                nc.scalar.activation(F[:, h:2 * h], T[:, h:2 * h], AFT.Exp)
            else:
                nc.scalar.activation(F[:, :], T[:, :], AFT.Exp)
            return F

        def stage_sadd(i, F):
            """DVE: sadd = f1 + f2 (bf16, 2x uop)."""
            s0, ns, b = TILES[i]
            Fv = F.rearrange("p (s l w) -> p s l w", s=ns, l=2)
            sf = mpool.tile([128, 4 * W], bf16, tag="sa",
                            name=f"sa_{i}", bufs=SA_BUFS)
            sadd = sf[:, 0:ns * W]
            sv = sadd.rearrange("p (s w) -> p s w", s=ns)
            k = ns - SADD_POOL_NS if ns == 4 else ns
            if k > 0:
                nc.vector.tensor_tensor(sv[:, 0:k, :], Fv[:, 0:k, 0, :],
                                        Fv[:, 0:k, 1, :], Alu.add)
            if k < ns:
                nc.gpsimd.tensor_tensor(sv[:, k:ns, :], Fv[:, k:ns, 0, :],
                                        Fv[:, k:ns, 1, :], Alu.add)
            return sadd

        def stage_mm(i, T):
            """DVE: either MM = max(t_other, 0) (4x), or margin mins
            m_l = min(t_l, t_l - t_other) for the m-path."""
            s0, ns, b = TILES[i]
            Tv = T.rearrange("p (s l w) -> p s l w", s=ns, l=2)
            MMf = mpool.tile([128, 4 * 2 * W], bf16, tag="MM",
                             name=f"MM_{i}", bufs=1)
            MM = MMf[:, 0:ns * 2 * W]
            MMv = MM.rearrange("p (l s w) -> p l s w", l=2, s=ns)
            if D_MPATH:
                UVf = mpool.tile([128, 4 * 2 * W], bf16, tag="UV",
                                 name=f"UV_{i}", bufs=1)
                u = UVf[:, 0:ns * W]
                v = UVf[:, 4 * W:4 * W + ns * W]
                uv = u.rearrange("p (s w) -> p s w", s=ns)
                vv = v.rearrange("p (s w) -> p s w", s=ns)
                if i < MPATH_SPLIT_FIRST:
                    for si in range(ns):
                        ssl = slice(si, si + 1)
                        nc.vector.tensor_tensor(uv[:, ssl, :],
                                                Tv[:, ssl, 0, :],
                                                Tv[:, ssl, 1, :],
                                                Alu.subtract)
                        nc.vector.tensor_scalar_mul(vv[:, ssl, :],
                                                    uv[:, ssl, :], -1.0)
                        nc.vector.tensor_tensor(MMv[:, 0, ssl, :],
                                                Tv[:, ssl, 0, :],
                                                uv[:, ssl, :], Alu.min)
                        nc.vector.tensor_tensor(MMv[:, 1, ssl, :],
                                                Tv[:, ssl, 1, :],
                                                vv[:, ssl, :], Alu.min)
                else:
                    nc.vector.tensor_tensor(uv[:, :, :], Tv[:, :, 0, :],
                                            Tv[:, :, 1, :], Alu.subtract)
                    nc.vector.tensor_scalar_mul(v[:, :], u[:, :], -1.0)
                    nc.vector.tensor_tensor(MMv[:, 0, :, :], Tv[:, :, 0, :],
                                            uv[:, :, :], Alu.min)
                    nc.vector.tensor_tensor(MMv[:, 1, :, :], Tv[:, :, 1, :],
                                            vv[:, :, :], Alu.min)
            else:
                nc.vector.tensor_scalar(MMv[:, 0, :, :], Tv[:, :, 1, :], 0.0,
                                        None, Alu.max)
                nc.vector.tensor_scalar(MMv[:, 1, :, :], Tv[:, :, 0, :], 0.0,
                                        None, Alu.max)
            return MM

        def stage_d(i, T, MM):
            """DVE: d-counts. MM-path: STT is_gt w/ accum (1x).
            m-path: tensor_scalar is_gt/add w/ accum (4x)."""
            s0, ns, b = TILES[i]
            Tv = T.rearrange("p (s l w) -> p s l w", s=ns, l=2)
            MMv = MM.rearrange("p (l s w) -> p l s w", l=2, s=ns)
            df = mpool.tile([128, 4 * 2 * W], bf16, tag="UV",
                            name=f"dsv_{i}", bufs=1)
            dscr = df[:, 0:ns * 2 * W]
            dv = dscr.rearrange("p (s l w) -> p s l w", s=ns, l=2)
            for si in range(ns):
                col = b * 8 + (s0 + si)
                for li, DM in ((0, DM1), (1, DM2)):
                    if D_MPATH:
                        nc.vector.tensor_scalar(
                            dv[:, si, li, :], MMv[:, li, si, :], 0.0, 0.0,
                            Alu.is_gt, Alu.add,
                            accum_out=DM[:, col:col + 1])
                    else:
                        nc.vector.scalar_tensor_tensor(
                            dv[:, si, li, :], Tv[:, si, li, :], 0.0,
                            MMv[:, li, si, :], Alu.add, Alu.is_gt,
                            accum_out=DM[:, col:col + 1])

        def stage_lnrexp(i, sadd):
            """r = 1/(1+f1+f2): ACT ln+exp, or DVE recip for RECIP_TILES."""
            s0, ns, b = TILES[i]
            lf = mpool.tile([128, 4 * W], bf16, tag="ln",
                            name=f"ln_{i}", bufs=2)
            lns = lf[:, 0:ns * W]
            rf = mpool.tile([128, 4 * W], bf16, tag="rb",
                            name=f"rb_{i}", bufs=RB_BUFS)
            rb = rf[:, 0:ns * W]
            if i in RECIP_TILES:
                nc.gpsimd.tensor_scalar_add(lns[:, :], sadd[:, :], 1.0)
                with nc.allow_low_precision(reason="bf16 softmax r"):
                    nc.vector.reciprocal(rb[:, :], lns[:, :])
            else:
                nc.scalar.activation(lns[:, :], sadd[:, :], AFT.Ln, bias=1.0)
                nc.scalar.activation(rb[:, :], lns[:, :], AFT.Exp, scale=-1.0)
            return rb

        def stage_prod(i, F, rb):
            """p2 = f2 * r only; p1-sums come from N - sum(r) - sum(p2)."""
            s0, ns, b = TILES[i]
            Fv = F.rearrange("p (s l w) -> p s l w", s=ns, l=2)
            rv = rb.rearrange("p (s w) -> p s w", s=ns)
            pf = mpool.tile([128, 4 * W], bf16, tag="pc",
                            name=f"pc_{i}", bufs=1)
            pscr = pf[:, 0:ns * W]
            pv = pscr.rearrange("p (s w) -> p s w", s=ns)
            if ns == 4:
                k = P2_DVE_NS
                if k > 0:
                    nc.vector.tensor_tensor(pv[:, 0:k, :],
                                            Fv[:, 0:k, 1, :],
                                            rv[:, 0:k, :], Alu.mult)
                nc.gpsimd.tensor_tensor(pv[:, k:4, :], Fv[:, k:4, 1, :],
                                        rv[:, k:4, :], Alu.mult)
            else:
                nc.vector.tensor_tensor(pv[:, :, :], Fv[:, :, 1, :],
                                        rv[:, :, :], Alu.mult)
            return pscr

        def stage_pe(i, rb, pscr):
            """PE: one-hot matmuls accumulate plane sums into PSUM.
            PS0 accumulates r (= p0), PS2 accumulates p2."""
            s0, ns, b = TILES[i]
            pv = pscr.rearrange("p (s w) -> p s w", s=ns)
            rv = rb.rearrange("p (s w) -> p s w", s=ns)
            for si in range(ns):
                s = s0 + si
                for src_v, PS in ((rv, PS0), (pv, PS2)):
                    k = mm_count[0]
                    mm_count[0] += 1
                    nc.tensor.matmul(PS[:, :], oh[:, 8 * s:8 * s + 8],
                                     src_v[:, si, :],
                                     start=(k < 2), stop=(k >= MM_TOTAL - 2))

        # ---- software-pipelined emission (2-tile skew) ----
        # ACT order per iteration: [ln/rexp for i-2, exp for i] so the
        # ln of tile i runs two iterations later than its sadd (no ACT
        # stall on the DVE round trip); products likewise at i-2.
        N = len(TILES)
        Xs = {}
        state = {}   # i -> (F, sadd)

        def drain_stage(j):
            Fm, saddm = state.pop(j)
            rbm = stage_lnrexp(j, saddm)
            pscr = stage_prod(j, Fm, rbm)
            stage_pe(j, rbm, pscr)

        for i in range(N):
            if i == 0:
                Xs[0] = stage_dma(0)
                Xs[1] = stage_dma(1)
            T = stage_sub(i, Xs[i])
            if i >= 2:
                drain_stage(i - 2)
            F = stage_exp(i, T)
            MM = stage_mm(i, T)
            sadd = stage_sadd(i, F)
            stage_d(i, T, MM)
            if i + 2 < N:
                Xs[i + 2] = stage_dma(i + 2)
            state[i] = (F, sadd)
        drain_stage(N - 2)
        drain_stage(N - 1)

        # ---- tail ----
        # const loads for the d-tail (deferred; they aren't needed earlier)
        nc.sync.dma_start(iota[:, :], iota_in[:, :])
        nc.sync.dma_start(ident[:, :], ident_in[:, :])
        O = cpool.tile([1, 40], f32, tag="O")

        # p-tail: PSUM [8,512] -> [8,1] sums (DVE + ACT in parallel), then
        # transpose to [1,8]
        ms0 = cpool.tile([8, 1], f32, tag="ms0")
        ms2 = cpool.tile([8, 1], f32, tag="ms2")
        msum = cpool.tile([8, 1], f32, tag="msum")
        ms2scr = cpool.tile([8, 512], f32, tag="ms2scr")
        nc.vector.tensor_reduce(ms0[:, :], PS0[:, :], X_AX, op=Alu.add)
        if MS2_DVE:
            nc.vector.tensor_reduce(ms2[:, :], PS2[:, :], X_AX, op=Alu.add)
        else:
            nc.scalar.activation(ms2scr[:, :], PS2[:, :], AFT.Copy,
                                 accum_out=ms2[:, :])
        nc.vector.tensor_tensor(msum[:, :], ms0[:, :], ms2[:, :], Alu.add)
        MT = ppool.tile([1, 16], f32, tag="MT")
        nc.tensor.transpose(MT[:, 0:8], msum[:, :], ident[0:8, 0:8])
        nc.tensor.transpose(MT[:, 8:16], ms2[:, :], ident[0:8, 0:8])
        sc = 1.0 / HW
        # rows: [cdr, disc=l2, cup=l1, disc, cup]; cup = 1 - (p0m + p2m)
        nc.vector.tensor_scalar(O[:, 8:16], MT[:, 8:16], sc, None, Alu.mult)
        nc.vector.tensor_scalar(O[:, 16:24], MT[:, 0:8], -sc, 1.0,
                                Alu.mult, Alu.add)
        nc.vector.tensor_scalar(O[:, 24:32], MT[:, 8:16], sc, None, Alu.mult)
        nc.vector.tensor_scalar(O[:, 32:40], MT[:, 0:8], -sc, 1.0,
                                Alu.mult, Alu.add)

        # d-tail: heights from DM1/DM2 (as in baseline)
        heights = []
        dparts = []
        for li, DM in enumerate((DM1, DM2)):
            TD = ppool.tile([32, 128], f32, tag="TD", name=f"TD{li}")
            nc.tensor.transpose(TD[:, :], DM[:, :], ident[:, :])
            pen = cpool.tile([32, 128], f32, tag=f"pen{li}")
            nc.vector.tensor_scalar(pen[:, :], TD[:, :], 0.5, 1e6,
                                    Alu.is_lt, Alu.mult)
            cmin = cpool.tile([32, 128], f32, tag=f"cmin{li}")
            nc.gpsimd.tensor_tensor(cmin[:, :], pen[:, :], iota[:, :], Alu.add)
            cmax = cpool.tile([32, 128], f32, tag=f"cmax{li}")
            nc.gpsimd.tensor_tensor(cmax[:, :], iota[:, :], pen[:, :],
                                    Alu.subtract)
            Y = cpool.tile([32, 2], f32, tag=f"Y{li}")
            nc.vector.tensor_reduce(Y[:, 0:1], cmin[:, :], X_AX, op=Alu.min)
            nc.vector.tensor_reduce(Y[:, 1:2], cmax[:, :], X_AX, op=Alu.max)
            YTmin = ppool.tile([1, 32], f32, tag="YTmin", name=f"YTmin{li}")
            YTmax = ppool.tile([1, 32], f32, tag="YTmax", name=f"YTmax{li}")
            nc.tensor.transpose(YTmin[:, :], Y[:, 0:1], ident[0:32, 0:32])
            nc.tensor.transpose(YTmax[:, :], Y[:, 1:2], ident[0:32, 0:32])
            ymin8 = cpool.tile([1, 8], f32, tag=f"ymin{li}")
            ymax8 = cpool.tile([1, 8], f32, tag=f"ymax{li}")
            nc.vector.tensor_reduce(
                ymin8[:, :], YTmin[0:1, :].rearrange("p (b s) -> p s b", b=4),
                X_AX, op=Alu.min)
            nc.vector.tensor_reduce(
                ymax8[:, :], YTmax[0:1, :].rearrange("p (b s) -> p s b", b=4),
                X_AX, op=Alu.max)
            hL = cpool.tile([1, 8], f32, tag=f"h{li}")
            nc.vector.tensor_tensor(hL[:, :], ymax8[:, :], ymin8[:, :],
                                    Alu.subtract)
            nc.vector.tensor_scalar_max(hL[:, :], hL[:, :], 0.0)
            heights.append(hL)

        h_cup, h_disc = heights
        den = cpool.tile([1, 8], f32, tag="den")
        nc.vector.tensor_scalar_add(den[:, :], h_disc[:, :], 1e-6)
        rec = cpool.tile([1, 8], f32, tag="rec")
        nc.vector.reciprocal(rec[:, :], den[:, :])
        nc.vector.tensor_tensor(O[:, 0:8], h_cup[:, :], rec[:, :], Alu.mult)

        nc.sync.dma_start(out[:, :], O[:, :])

    nc.finalize()
    return nc


def _get_nc():
    if "nc" not in _CACHE:
        _CACHE["nc"] = _build()
    return _CACHE["nc"]


def _host_inputs():
    iota = (np.arange(128, dtype=np.float32)[None, :]
            + 128.0 * np.repeat(np.arange(4, dtype=np.float32), 8)[:, None])
    ident = np.eye(128, dtype=np.float32)
    # oh[:, 8s+j] = 1 iff j == s (one-hot stationary for per-sample matmul)
    import ml_dtypes
    oh = np.zeros((128, 64), dtype=ml_dtypes.bfloat16)
    for s in range(8):
        oh[:, 8 * s + s] = 1.0
    return iota, ident, oh


def _run(seg_mask, trace=False):
    from concourse.bass_utils import run_bass_kernel_spmd

    x = np.ascontiguousarray(np.asarray(seg_mask, dtype=np.float32))
    assert x.shape == (B, C, H, W)
    iota, ident, oh = _host_inputs()
    in_maps = [
        {"x": x[SPC * c:SPC * (c + 1)], "iota": iota, "ident": ident,
         "oh": oh}
        for c in range(NCORES)
    ]
    nc = _get_nc()
    res = run_bass_kernel_spmd(nc, in_maps, core_ids=list(range(NCORES)),
                               trace=trace)
    outs = []
    for c in range(NCORES):
        o = np.asarray(res.results[c]["out"]).reshape(5, SPC).T
        outs.append(o)
    full = np.concatenate(outs, axis=0).astype(np.float32)
    return full, res


def kernel(segmentation_mask):
    full, _ = _run(segmentation_mask, trace=False)
    return full
